# revision 3
# baseline (speedup 1.0000x reference)
"""Trainium2 Bass kernel for nn_Attention_5935644803277 (CvT-style sparse attention).

Full-input contract: kernel(**inputs) takes the unsharded inputs (x: [32,1536,768])
and returns the full output [32,1536,768]. Internally shards batch 32 -> 4 per core
across 8 NeuronCores (SPMD, no collectives).

Math (per batch):
  tpl = x[:256] as 16x16 image, onl = x[256:512] as 16x16, srch = x[512:] as 32x32
  q = concat(dwconv3x3_s1(img) for img) -> BN -> @ wq.T   (1536 tokens)
  k,v = same with stride 2 -> 384 tokens
  heads(12, hd=64); templates (first 512 q) attend to first 128 k/v;
  search (last 1024 q) attend to all 384; softmax(QK^T * 768^-0.5);
  out = concat @ w_proj.T + b_proj

Kernel-side simplifications (host-precomputed):
  - BN folded into projection weights: W*_eff[c,d] = w*[d,c] * inv*[c]
  - K-projection bias dropped (softmax shift invariance)
  - V-projection bias folded into final bias: b_fin = b_proj + w_proj @ (wv @ beta_v)
  - Q-projection bias bq_eff = wq @ beta_q applied at PSUM evacuation
  - softmax denominator via ones-column appended to V stationary
"""
import numpy as np

import concourse.bass as bass
import concourse.tile as tile
from concourse import bacc, mybir
from concourse.bass_utils import run_bass_kernel_spmd

F32 = mybir.dt.float32
BF16 = mybir.dt.bfloat16
AF = mybir.ActivationFunctionType
OP = mybir.AluOpType

EPS = 1e-5
NB = 4          # batches per core
L = 1536
D = 768
G = 6           # channel chunks of 128
NH = 12
HD = 64
SCALE = float(D) ** -0.5
LKV = 384


def _rect(tile_ap, base, dims):
    """AP at tile's partition dim + given free-dim [step,count] list, at free offset base."""
    return bass.AP(tensor=tile_ap.tensor, offset=tile_ap.offset + base,
                   ap=[list(tile_ap.ap[0])] + [list(d) for d in dims])


def _tap_bounds(d, H):
    """stride-1 output row range for tap offset d in {0,1,2}."""
    r0 = 1 if d == 0 else 0
    r1 = H - 2 if d == 2 else H - 1
    return r0, r1 - r0 + 1


def _tap_bounds_s2(d, H):
    """stride-2: output rows where input row 2r+d-1 in [0,H). H even."""
    Ho = H // 2
    r0 = 1 if d == 0 else 0
    return r0, Ho - r0


def build_program():
    nc = bacc.Bacc("TRN2", target_bir_lowering=False, debug=False, num_devices=8)

    x_d = nc.dram_tensor("x", [NB, L, D], F32, kind="ExternalInput").ap()
    w_d = {n: nc.dram_tensor(n, [D, D], F32, kind="ExternalInput").ap()
           for n in ("wq", "wk", "wv", "wp")}
    bq_d = nc.dram_tensor("bq", [128, G], F32, kind="ExternalInput").ap()
    bfin_d = nc.dram_tensor("bfin", [1, D], F32, kind="ExternalInput").ap()
    cw_d = {n: nc.dram_tensor(n, [128, G, 9], F32, kind="ExternalInput").ap()
            for n in ("cwq", "cwk", "cwv")}
    ident_d = nc.dram_tensor("ident", [128, 128], F32, kind="ExternalInput").ap()
    ones12_d = nc.dram_tensor("ones12", [128, NH], F32, kind="ExternalInput").ap()
    ones1_d = nc.dram_tensor("ones1", [1, 128], F32, kind="ExternalInput").ap()
    out_d = nc.dram_tensor("out", [NB, L, D], F32, kind="ExternalOutput").ap()

    with tile.TileContext(nc) as tc:
        with (
            tc.tile_pool(name="consts", bufs=1) as consts,
            tc.tile_pool(name="wpool", bufs=1) as wpool,
            tc.tile_pool(name="act", bufs=1) as actp,       # per-batch activations
            tc.tile_pool(name="roll", bufs=1) as roll,      # rotating small tiles
            tc.tile_pool(name="ps_a", bufs=2, space="PSUM") as ps_a,    # transposes + scores
            tc.tile_pool(name="ps_pj", bufs=2, space="PSUM") as ps_pj,  # projections
            tc.tile_pool(name="ps_kv", bufs=2, space="PSUM") as ps_kv,  # kv-conv
            tc.tile_pool(name="ps_u", bufs=2, space="PSUM") as ps_u,    # A@V | sums
        ):
            # ---------------- constants ----------------
            ident_sb = consts.tile([128, 128], F32, name="ident_sb")
            nc.sync.dma_start(out=ident_sb, in_=ident_d)
            bq_sb = consts.tile([128, G], F32, name="bq_sb")
            nc.sync.dma_start(out=bq_sb, in_=bq_d)
            ones12_sb = consts.tile([128, NH], F32, name="ones12_sb")
            nc.sync.dma_start(out=ones12_sb, in_=ones12_d)
            bfin_f = consts.tile([1, D], F32, name="bfin_f")
            nc.sync.dma_start(out=bfin_f, in_=bfin_d)
            bfin_sb = consts.tile([1, D], BF16, name="bfin_sb")
            nc.scalar.copy(bfin_sb, bfin_f)
            ones1_f = consts.tile([1, 128], F32, name="ones1_f")
            nc.sync.dma_start(out=ones1_f, in_=ones1_d)
            ones1_sb = consts.tile([1, 128], BF16, name="ones1_sb")
            nc.scalar.copy(ones1_sb, ones1_f)
            ones768_sb = consts.tile([128, D], F32, name="ones768_sb")
            nc.vector.memset(ones768_sb, 1.0)
            cw_sb = {}
            for n in ("cwq", "cwk", "cwv"):
                cw_sb[n] = consts.tile([128, G, 9], F32, name=f"{n}_sb")
                nc.sync.dma_start(out=cw_sb[n], in_=cw_d[n])
            w_sb = {}
            for n in ("wq", "wk", "wv", "wp"):
                w_sb[n] = wpool.tile([128, G, D], BF16, name=f"{n}_sb")
                for g in range(G):
                    wst = roll.tile([128, D], F32, name=f"wst_{n}_{g}",
                                    tag="wst", bufs=2)
                    nc.sync.dma_start(out=wst, in_=w_d[n][g * 128:(g + 1) * 128, :])
                    nc.scalar.copy(w_sb[n][:, g, :], wst)
            # diag stationaries for PE k/v conv: diags[:, c, g, t, :]
            diags = consts.tile([128, 2, G, 9, 128], BF16, name="diags")
            for ci, n in enumerate(("cwk", "cwv")):
                for g in range(G):
                    for t in range(9):
                        nc.vector.tensor_scalar_mul(
                            diags[:, ci, g, t, :], ident_sb, cw_sb[n][:, g, t:t + 1])

            # ---------------- per batch ----------------
            for b in range(NB):
                # ---- phase T: transpose x[b] -> xt (channels on partitions, bf16)
                xt = actp.tile([128, G, L], BF16, name=f"xt_{b}", tag="xt")
                for k3 in range(3):  # 512-token slabs
                    xn = roll.tile([128, 4, D], F32, name=f"xn_{b}_{k3}",
                                   tag="xn", bufs=2)
                    nc.sync.dma_start(
                        out=xn,
                        in_=x_d[b, k3 * 512:(k3 + 1) * 512, :].rearrange(
                            "(m p) d -> p m d", p=128))
                    for g in range(G):
                        tp = ps_a.tile([128, 512], F32, name=f"tp_{b}_{k3}_{g}",
                                       tag="a512")
                        for j in range(4):
                            nc.tensor.transpose(
                                tp[:, j * 128:(j + 1) * 128],
                                xn[:, j, g * 128:(g + 1) * 128], ident_sb)
                        nc.scalar.copy(xt[:, g, k3 * 512:(k3 + 1) * 512], tp)

                # ---- phase conv-q (DVE scalar_tensor_tensor FMA)
                cq = actp.tile([128, G, L], BF16, name=f"cq_{b}", tag="cq")
                cwq = cw_sb["cwq"]
                for g in range(G):
                    gb = g * L
                    # center tap first (full coverage init)
                    nc.vector.tensor_scalar_mul(
                        cq[:, g, 0:512], xt[:, g, 0:512], cwq[:, g, 4:5])
                    nc.vector.tensor_scalar_mul(
                        cq[:, g, 512:L], xt[:, g, 512:L], cwq[:, g, 4:5])
                    for dh in range(3):
                        for dw in range(3):
                            if dh == 1 and dw == 1:
                                continue
                            sc = cwq[:, g, 3 * dh + dw:3 * dh + dw + 1]
                            # tpl and onl (16x16 images at 0 and 256); STT APs
                            # are limited to 3D, so one op per image
                            r0, nr = _tap_bounds(dh, 16)
                            c0, nw = _tap_bounds(dw, 16)
                            for ib in (0, 256):
                                ia = _rect(xt, gb + ib + (r0 + dh - 1) * 16 + (c0 + dw - 1),
                                           [[16, nr], [1, nw]])
                                oa = _rect(cq, gb + ib + r0 * 16 + c0,
                                           [[16, nr], [1, nw]])
                                nc.vector.scalar_tensor_tensor(
                                    out=oa, in0=ia, scalar=sc, in1=oa,
                                    op0=OP.mult, op1=OP.add)
                            # srch (32x32 at 512)
                            r0, nr = _tap_bounds(dh, 32)
                            c0, nw = _tap_bounds(dw, 32)
                            ia = _rect(xt, gb + 512 + (r0 + dh - 1) * 32 + (c0 + dw - 1),
                                       [[32, nr], [1, nw]])
                            oa = _rect(cq, gb + 512 + r0 * 32 + c0,
                                       [[32, nr], [1, nw]])
                            nc.vector.scalar_tensor_tensor(
                                out=oa, in0=ia, scalar=sc, in1=oa,
                                op0=OP.mult, op1=OP.add)

                # ---- phase conv-k/v (PE diag matmuls)
                ckv = actp.tile([128, 2, G, LKV], BF16, name=f"ckv_{b}", tag="ckv")
                for ci in range(2):
                    for g in range(G):
                        kvp = ps_kv.tile([128, LKV], F32,
                                         name=f"kvp_{b}_{ci}_{g}", tag="kvps")
                        gb = g * L
                        taps = [(1, 1)] + [(dh, dw) for dh in range(3)
                                           for dw in range(3) if (dh, dw) != (1, 1)]
                        n_mm = 2 * len(taps)
                        mm_i = 0
                        for dh, dw in taps:
                            dg = diags[:, ci, g, 3 * dh + dw, :]
                            # tpl + onl (16x16 -> 8x8 at out 0,64)
                            r0, nr = _tap_bounds_s2(dh, 16)
                            c0, nw = _tap_bounds_s2(dw, 16)
                            ia = _rect(xt, gb + (2 * r0 + dh - 1) * 16 + (2 * c0 + dw - 1),
                                       [[256, 2], [32, nr], [2, nw]])
                            oa = _rect(kvp, r0 * 8 + c0, [[64, 2], [8, nr], [1, nw]])
                            nc.tensor.matmul(oa, dg, ia, start=(mm_i == 0),
                                             stop=(mm_i == n_mm - 1),
                                             skip_group_check=True)
                            mm_i += 1
                            # srch (32x32 -> 16x16 at out 128)
                            r0, nr = _tap_bounds_s2(dh, 32)
                            c0, nw = _tap_bounds_s2(dw, 32)
                            ia = _rect(xt, gb + 512 + (2 * r0 + dh - 1) * 32 + (2 * c0 + dw - 1),
                                       [[64, nr], [2, nw]])
                            oa = _rect(kvp, 128 + r0 * 16 + c0, [[16, nr], [1, nw]])
                            nc.tensor.matmul(oa, dg, ia, start=False,
                                             stop=(mm_i == n_mm - 1),
                                             skip_group_check=True)
                            mm_i += 1
                        nc.scalar.copy(ckv[:, ci, g, :], kvp)

                # ---- K projection: kt[d_chunk, key] (transposed layout)
                kt = actp.tile([128, G, LKV], BF16, name=f"kt_{b}", tag="kt")
                for go in range(G):
                    pj = ps_pj.tile([128, LKV], F32, name=f"kpj_{b}_{go}", tag="pj")
                    for g in range(G):
                        nc.tensor.matmul(
                            pj, w_sb["wk"][:, g, go * 128:(go + 1) * 128],
                            ckv[:, 0, g, :], start=(g == 0), stop=(g == G - 1))
                    nc.scalar.copy(kt[:, go, :], pj)

                # ---- V projection: natural layout [key, d] + 64-wide ones block
                # (stationary [V_h | ones(64)] makes A@V produce the softmax sums
                #  replicated across PSUM partitions 64..127 in the same matmul)
                v_sb = actp.tile([128, 3, NH, 2 * HD], BF16, name=f"v_{b}", tag="v")
                for mt in range(3):
                    oa = _rect(v_sb, mt * NH * 2 * HD + HD, [[2 * HD, NH], [1, HD]])
                    nc.scalar.copy(oa, ones768_sb)
                    for nh in range(2):
                        pj = ps_pj.tile([128, LKV], F32,
                                        name=f"vpj_{b}_{mt}_{nh}", tag="pj")
                        for g in range(G):
                            nc.tensor.matmul(
                                pj, ckv[:, 1, g, mt * 128:(mt + 1) * 128],
                                w_sb["wv"][:, g, nh * 384:(nh + 1) * 384],
                                start=(g == 0), stop=(g == G - 1))
                        oa = _rect(v_sb, mt * NH * 2 * HD + nh * 6 * 2 * HD,
                                   [[2 * HD, 6], [1, HD]])
                        nc.scalar.copy(oa, pj)

                # ---- attention per 512-query tile
                for t in range(3):
                    qt = roll.tile([128, G, 512], BF16, name=f"qt_{b}_{t}",
                                   tag="qt", bufs=2)
                    for go in range(G):
                        pj = ps_pj.tile([128, 512], F32,
                                        name=f"qpj_{b}_{t}_{go}", tag="pj")
                        for g in range(G):
                            nc.tensor.matmul(
                                pj, w_sb["wq"][:, g, go * 128:(go + 1) * 128],
                                cq[:, g, t * 512:(t + 1) * 512],
                                start=(g == 0), stop=(g == G - 1))
                        nc.scalar.activation(qt[:, go, :], pj, AF.Identity,
                                             bias=bq_sb[:, go:go + 1])

                    xatt = roll.tile([128, G, 512], BF16, name=f"xatt_{b}_{t}",
                                     tag="xatt", bufs=2)
                    kcs = (0,) if t == 0 else (0, 1, 2)
                    for h in range(NH):
                        g, po = h // 2, (h % 2) * HD
                        ups = ps_u.tile([128, 512], F32,
                                        name=f"u_{b}_{t}_{h}", tag="u")
                        for i, kc in enumerate(kcs):
                            sps = ps_a.tile([128, 512], F32,
                                            name=f"s_{b}_{t}_{h}_{kc}", tag="a512")
                            nc.tensor.matmul(
                                sps, kt[po:po + HD, g, kc * 128:(kc + 1) * 128],
                                qt[po:po + HD, g, :], start=True, stop=True)
                            aT = roll.tile([128, 512], BF16,
                                           name=f"aT_{b}_{t}_{h}_{kc}",
                                           tag="aT", bufs=3)
                            nc.scalar.activation(aT, sps, AF.Exp, scale=SCALE)
                            nc.tensor.matmul(
                                ups, v_sb[:, kc, h, :], aT,
                                start=(i == 0), stop=(i == len(kcs) - 1))
                        recip = roll.tile([HD, 512], F32, name=f"rc_{b}_{t}_{h}",
                                          tag="recip", bufs=2)
                        nc.vector.reciprocal(recip, ups[HD:2 * HD, :])
                        nc.vector.tensor_mul(xatt[po:po + HD, g, :],
                                             ups[0:HD, :], recip)

                    # ---- output projection (natural layout) + bias + store
                    for mt2 in range(4):
                        onat = roll.tile([128, D], F32, name=f"on_{b}_{t}_{mt2}",
                                         tag="onat", bufs=3)
                        for nh in range(2):
                            pj = ps_pj.tile([128, 384], F32,
                                            name=f"opj_{b}_{t}_{mt2}_{nh}", tag="pj")
                            for g in range(G):
                                nc.tensor.matmul(
                                    pj, xatt[:, g, mt2 * 128:(mt2 + 1) * 128],
                                    w_sb["wp"][:, g, nh * 384:(nh + 1) * 384],
                                    start=(g == 0), stop=False)
                            nc.tensor.matmul(
                                pj, ones1_sb, bfin_sb[:, nh * 384:(nh + 1) * 384],
                                start=False, stop=True)
                            nc.scalar.copy(onat[:, nh * 384:(nh + 1) * 384], pj)
                        tok0 = t * 512 + mt2 * 128
                        nc.sync.dma_start(out=out_d[b, tok0:tok0 + 128, :], in_=onat)

    nc.compile()
    return nc


_NC_CACHE = {}


def _get_program():
    if "nc" not in _NC_CACHE:
        _NC_CACHE["nc"] = build_program()
    return _NC_CACHE["nc"]


def _host_prep(inputs):
    f = lambda k: np.asarray(inputs[k], dtype=np.float32)
    w = {}
    effs = {}
    for n in ("q", "k", "v"):
        inv = f(f"bn_{n}_g") / np.sqrt(f(f"bn_{n}_v") + EPS)
        beta = f(f"bn_{n}_b") - f(f"bn_{n}_m") * inv
        effs[n] = (inv, beta)
    wq, wk, wv, wp = f("wq"), f("wk"), f("wv"), f("w_proj")
    w["wq"] = np.ascontiguousarray((wq * effs["q"][0][None, :]).T)
    w["wk"] = np.ascontiguousarray((wk * effs["k"][0][None, :]).T)
    w["wv"] = np.ascontiguousarray((wv * effs["v"][0][None, :]).T)
    w["wp"] = np.ascontiguousarray(wp.T)
    bq_eff = wq @ effs["q"][1]
    bv_eff = wv @ effs["v"][1]
    b_fin = f("b_proj") + wp @ bv_eff
    w["bq"] = np.ascontiguousarray(bq_eff.reshape(G, 128).T)
    w["bfin"] = b_fin.reshape(1, D)
    for n, key in (("cwq", "conv_q_w"), ("cwk", "conv_k_w"), ("cwv", "conv_v_w")):
        cw = f(key).reshape(D, 9)
        w[n] = np.ascontiguousarray(cw.reshape(G, 128, 9).transpose(1, 0, 2))
    w["ident"] = np.eye(128, dtype=np.float32)
    w["ones12"] = np.ones((128, NH), np.float32)
    w["ones1"] = np.ones((1, 128), np.float32)
    return {k: np.ascontiguousarray(v, dtype=np.float32) for k, v in w.items()}


def kernel(**inputs):
    x = np.asarray(inputs["x"], dtype=np.float32)
    B = x.shape[0]
    assert x.shape == (32, L, D), x.shape
    const = _host_prep(inputs)
    nc = _get_program()
    in_maps = []
    for c in range(8):
        m = dict(const)
        m["x"] = np.ascontiguousarray(x[c * NB:(c + 1) * NB])
        in_maps.append(m)
    res = run_bass_kernel_spmd(nc, in_maps, list(range(8)))
    out = np.concatenate([res.results[c]["out"] for c in range(8)], axis=0)
    return out.astype(np.float32)
